# revision 1
# baseline (speedup 1.0000x reference)
"""CALayer (squeeze-excite channel attention) Bass/Tile kernel for Trainium2.

Problem: x[32, 512, 56, 56] f32
  pooled = mean(x, spatial)                       # [N, C]
  h  = ELU(GN1(pooled @ w1.T + b1))               # [N, 64]
  g  = sigmoid(GN2(h @ w2.T + b2))                # [N, C]
  out = x * g[:, :, None, None]

Sharding: data-parallel over batch — 4 images per core on 8 NeuronCores,
params replicated. Per core the kernel is memory-bound: stream 4x512x3136
f32 in (~24.5 MiB), reduce over the free dim for the pooled sums, run the
tiny per-image MLP, then rescale the (still SBUF-resident) tiles by the
per-(image,channel) gate and stream them back out.

Layout per image n (channel blocks cb of 128):
  x tile   [128, 3136]  partition=channel, free=spatial  (contiguous DRAM)
  pooled   [128, 4]     col cb = spatial sums of block cb
  h        [1, 64]      via 4 accumulating matmuls lhsT=pooled[:,cb] rhs=w1T_cb
  g        [1, 512]     via matmul lhsT=hT[64,1] rhs=w2T[64,512]
  gateT    [128, 4]     col cb = sigmoid(g) transposed back to partitions
"""

import numpy as np
from contextlib import ExitStack

import concourse.tile as tile
from concourse import bacc, masks, mybir
from concourse.bass_utils import run_bass_kernel_spmd

AF = mybir.ActivationFunctionType
ALU = mybir.AluOpType
AX = mybir.AxisListType
F32 = mybir.dt.float32

N_CORES = 8
N_PER_CORE = 4          # batch 32 / 8 cores
C = 512                 # channels
R = 64                  # squeezed channels (C // 8)
S = 56 * 56             # spatial size
P = 128                 # SBUF partitions
NCB = C // P            # channel blocks per image
EPS = 1e-5


def _groupnorm_row(nc, pool, t, w_sb, b_sb, eps_sb, tag):
    """GroupNorm(1 group) along the free dim of a [1, d] tile, in place."""
    stats = pool.tile([1, nc.vector.BN_STATS_DIM], F32, tag=f"bnst_{tag}")
    nc.vector.bn_stats(out=stats[:], in_=t[:])
    mv = pool.tile([1, nc.vector.BN_AGGR_DIM], F32, tag=f"bnmv_{tag}")
    nc.vector.bn_aggr(out=mv[:], in_=stats[:])
    # mv[:, 0] = mean, mv[:, 1] = var  ->  mv[:, 1] = 1/sqrt(var + eps)
    nc.scalar.activation(out=mv[:, 1:2], in_=mv[:, 1:2], func=AF.Sqrt,
                         bias=eps_sb[:], scale=1.0)
    nc.vector.reciprocal(out=mv[:, 1:2], in_=mv[:, 1:2])
    nc.vector.tensor_scalar(out=t[:], in0=t[:],
                            scalar1=mv[:, 0:1], scalar2=mv[:, 1:2],
                            op0=ALU.subtract, op1=ALU.mult)
    nc.vector.tensor_mul(out=t[:], in0=t[:], in1=w_sb[:])
    nc.vector.tensor_add(out=t[:], in0=t[:], in1=b_sb[:])


def _emit(ctx, tc, d, reps=1):
    nc = tc.nc
    singles = ctx.enter_context(tc.tile_pool(name="singles", bufs=1))
    xpool = ctx.enter_context(tc.tile_pool(name="xp", bufs=14))
    small = ctx.enter_context(tc.tile_pool(name="small", bufs=4))
    psum = ctx.enter_context(tc.tile_pool(name="psum", bufs=2, space="PSUM"))

    ident = singles.tile([P, P], F32, tag="ident")
    masks.make_identity(nc, ident[:])

    # prime the DMA queues with the first image's bulk loads so the stream
    # starts immediately; the tiny param DMAs below slot in right after.
    first_xts = []
    for cb in range(NCB):
        xt = xpool.tile([P, S], F32, tag="xt")
        with tc.high_priority():
            nc.sync.dma_start(out=xt[:], in_=d["x"][cb * P:(cb + 1) * P, :])
        first_xts.append(xt)

    # ---- replicated params: load + transpose the tiny 1x1-conv weights ----
    # w1T_cb [128, 64] = w1[:, cb*128:(cb+1)*128].T, pre-scaled by 1/S so the
    # pooled *sums* (not means) can feed the matmul directly.
    w1_sb = singles.tile([R, C], F32, tag="w1_sb")
    with tc.high_priority():
        nc.sync.dma_start(out=w1_sb[:], in_=d["w1"][:, :])
    w1T = []
    for cb in range(NCB):
        pst = psum.tile([P, R], F32, tag="wt")
        nc.tensor.transpose(pst[:], w1_sb[:, cb * P:(cb + 1) * P], ident[:R, :R])
        t = singles.tile([P, R], F32, tag=f"w1T{cb}")
        nc.scalar.mul(out=t[:], in_=pst[:], mul=1.0 / S)
        w1T.append(t)

    # w2T [64, 512] = w2.T
    w2T = singles.tile([R, C], F32, tag="w2T")
    for cb in range(NCB):
        w2_sb = small.tile([P, R], F32, tag="w2ld")
        with tc.high_priority():
            nc.sync.dma_start(out=w2_sb[:], in_=d["w2"][cb * P:(cb + 1) * P, :])
        pst = psum.tile([R, P], F32, tag="wt")
        nc.tensor.transpose(pst[:], w2_sb[:], ident[:P, :P])
        nc.vector.tensor_copy(out=w2T[:, cb * P:(cb + 1) * P], in_=pst[:])

    def vec_row(name, width):
        t = singles.tile([1, width], F32, tag=name)
        with tc.high_priority():
            nc.sync.dma_start(out=t[:], in_=d[name][None, :])
        return t

    b1_sb = vec_row("b1", R)
    g1w_sb = vec_row("gn1_w", R)
    g1b_sb = vec_row("gn1_b", R)
    b2_sb = vec_row("b2", C)
    g2w_sb = vec_row("gn2_w", C)
    g2b_sb = vec_row("gn2_b", C)
    eps_sb = singles.tile([1, 1], F32, tag="eps")
    nc.vector.memset(eps_sb[:], EPS)

    x_d, out_d = d["x"], d["out"]

    for it, n in enumerate([i for _ in range(reps) for i in range(N_PER_CORE)]):
        # ---- load + spatial-sum the 4 channel blocks of image n ----
        pooled = small.tile([P, NCB], F32, tag="pooled")
        if it == 0:
            xts = first_xts
        else:
            xts = []
            for cb in range(NCB):
                xt = xpool.tile([P, S], F32, tag="xt")
                r0 = n * C + cb * P
                # loads must win the DMA queues over stores: a store-interleaved
                # schedule delays the last image's loads (and so its gate) to
                # the very end, leaving the DMA pipe idle for the gate-latency.
                with tc.high_priority():
                    nc.sync.dma_start(out=xt[:], in_=x_d[r0:r0 + P, :])
                xts.append(xt)
        for cb in range(NCB):
            nc.vector.tensor_reduce(out=pooled[:, cb:cb + 1], in_=xts[cb][:],
                                    axis=AX.X, op=ALU.add)

        # ---- h = pooled_mean @ w1.T + b1 (1/S folded into w1T) ----
        psum_h = psum.tile([1, R], F32, tag="mm")
        for cb in range(NCB):
            nc.tensor.matmul(psum_h[:], lhsT=pooled[:, cb:cb + 1],
                             rhs=w1T[cb][:], start=(cb == 0), stop=(cb == NCB - 1))
        h = small.tile([1, R], F32, tag="h")
        nc.vector.tensor_add(out=h[:], in0=psum_h[:], in1=b1_sb[:])

        _groupnorm_row(nc, small, h, g1w_sb, g1b_sb, eps_sb, tag="1")

        # ELU(x) = max(x,0) + exp(min(x,0)) - 1
        tneg = small.tile([1, R], F32, tag="tneg")
        nc.vector.tensor_scalar_min(out=tneg[:], in0=h[:], scalar1=0.0)
        texp = small.tile([1, R], F32, tag="texp")
        nc.scalar.activation(out=texp[:], in_=tneg[:], func=AF.Exp)
        tpos = small.tile([1, R], F32, tag="tpos")
        nc.vector.tensor_scalar_max(out=tpos[:], in0=h[:], scalar1=0.0)
        nc.vector.tensor_add(out=h[:], in0=tpos[:], in1=texp[:])
        nc.vector.tensor_scalar_add(out=h[:], in0=h[:], scalar1=-1.0)

        # ---- g = h @ w2.T + b2 ----
        pst_h = psum.tile([R, 1], F32, tag="tp")
        nc.tensor.transpose(pst_h[:], h[:], ident[:1, :1])
        hT = small.tile([R, 1], F32, tag="hT")
        nc.vector.tensor_copy(out=hT[:], in_=pst_h[:])

        psum_g = psum.tile([1, C], F32, tag="mm")
        nc.tensor.matmul(psum_g[:], lhsT=hT[:], rhs=w2T[:], start=True, stop=True)
        g = small.tile([1, C], F32, tag="g")
        nc.vector.tensor_add(out=g[:], in0=psum_g[:], in1=b2_sb[:])

        _groupnorm_row(nc, small, g, g2w_sb, g2b_sb, eps_sb, tag="2")
        nc.scalar.activation(out=g[:], in_=g[:], func=AF.Sigmoid)

        # ---- gate back to partition layout, rescale, store ----
        gateT = small.tile([P, NCB], F32, tag="gateT")
        for cb in range(NCB):
            pst_g = psum.tile([P, 1], F32, tag="tp")
            nc.tensor.transpose(pst_g[:], g[:, cb * P:(cb + 1) * P], ident[:1, :1])
            nc.vector.tensor_copy(out=gateT[:, cb:cb + 1], in_=pst_g[:])
        for cb in range(NCB):
            # DVE f32 tensor_scalar gets 2x mode (1.7us/tile vs 2.8us on ACT)
            # and keeping Copy off ACT avoids LUT reload thrash between the
            # Sqrt/Exp/Sigmoid of consecutive images' MLP chains.
            nc.vector.tensor_scalar_mul(out=xts[cb][:], in0=xts[cb][:],
                                        scalar1=gateT[:, cb:cb + 1])
            r0 = n * C + cb * P
            nc.sync.dma_start(out=out_d[r0:r0 + P, :], in_=xts[cb][:])


def _build_program(reps=1):
    nc = bacc.Bacc("TRN2", target_bir_lowering=False, debug=False,
                   num_devices=N_CORES)
    d = {}
    d["x"] = nc.dram_tensor("x", [N_PER_CORE * C, S], F32,
                            kind="ExternalInput").ap()
    d["w1"] = nc.dram_tensor("w1", [R, C], F32, kind="ExternalInput").ap()
    d["b1"] = nc.dram_tensor("b1", [R], F32, kind="ExternalInput").ap()
    d["gn1_w"] = nc.dram_tensor("gn1_w", [R], F32, kind="ExternalInput").ap()
    d["gn1_b"] = nc.dram_tensor("gn1_b", [R], F32, kind="ExternalInput").ap()
    d["w2"] = nc.dram_tensor("w2", [C, R], F32, kind="ExternalInput").ap()
    d["b2"] = nc.dram_tensor("b2", [C], F32, kind="ExternalInput").ap()
    d["gn2_w"] = nc.dram_tensor("gn2_w", [C], F32, kind="ExternalInput").ap()
    d["gn2_b"] = nc.dram_tensor("gn2_b", [C], F32, kind="ExternalInput").ap()
    d["out"] = nc.dram_tensor("out", [N_PER_CORE * C, S], F32,
                              kind="ExternalOutput").ap()

    with tile.TileContext(nc) as tc:
        with ExitStack() as ctx:
            _emit(ctx, tc, d, reps=reps)
    nc.compile()
    return nc


_PROGS = {}


def _get_program(reps=1):
    if reps not in _PROGS:
        _PROGS[reps] = _build_program(reps=reps)
    return _PROGS[reps]


def _run(trace=False, **inputs):
    """Reference dispatch path via run_bass_kernel_spmd (host-copies the
    shards each call; kept as the non-axon-compatible fallback)."""
    nc = _get_program()
    x = np.ascontiguousarray(inputs["x"], dtype=np.float32)
    shards = x.reshape(N_CORES, N_PER_CORE * C, S)
    base = {k: np.ascontiguousarray(inputs[k], dtype=np.float32)
            for k in ("w1", "b1", "gn1_w", "gn1_b", "w2", "b2", "gn2_w", "gn2_b")}
    in_maps = [dict(base, x=shards[i]) for i in range(N_CORES)]
    res = run_bass_kernel_spmd(nc, in_maps, list(range(N_CORES)), trace=trace)
    out = np.concatenate(
        [r["out"].reshape(N_PER_CORE, C, 56, 56) for r in res.results], axis=0)
    return out, res


_RUNNER = None


def _get_runner():
    """Cached jitted SPMD dispatch (axon/PJRT): one bass_exec under a
    shard_map, compiled once. Feeding the global [8*2048, 3136] array avoids
    the per-call host shard-concat, and donation zeros are created on-device."""
    global _RUNNER
    if _RUNNER is not None:
        return _RUNNER
    import jax
    import jax.numpy as jnp
    from jax.sharding import Mesh, PartitionSpec, NamedSharding
    from jax.experimental.shard_map import shard_map
    from concourse.bass2jax import (
        _bass_exec_p, install_neuronx_cc_hook, partition_id_tensor)

    nc = _get_program()
    install_neuronx_cc_hook()
    partition_name = (nc.partition_id_tensor.name
                      if nc.partition_id_tensor else None)
    in_names, out_names, out_avals = [], [], []
    for alloc in nc.m.functions[0].allocations:
        if not isinstance(alloc, mybir.MemoryLocationSet):
            continue
        name = alloc.memorylocations[0].name
        if alloc.kind == "ExternalInput":
            if name != partition_name:
                in_names.append(name)
        elif alloc.kind == "ExternalOutput":
            out_names.append(name)
            out_avals.append(jax.core.ShapedArray(
                tuple(alloc.tensor_shape), mybir.dt.np(alloc.dtype)))
    all_in_names = tuple(in_names + out_names)
    if partition_name is not None:
        all_in_names = all_in_names + (partition_name,)

    def _body(*args):
        operands = list(args)
        if partition_name is not None:
            operands.append(partition_id_tensor())
        return tuple(_bass_exec_p.bind(
            *operands,
            out_avals=tuple(out_avals),
            in_names=all_in_names,
            out_names=tuple(out_names),
            lowering_input_output_aliases=(),
            sim_require_finite=True,
            sim_require_nnan=True,
            nc=nc,
        ))

    mesh = Mesh(np.asarray(jax.devices()[:N_CORES]), ("core",))
    nspec = (PartitionSpec("core"),)
    n_in = len(in_names)
    n_out = len(out_names)
    fn = jax.jit(
        shard_map(_body, mesh=mesh, in_specs=nspec * (n_in + n_out),
                  out_specs=nspec * n_out, check_rep=False),
        donate_argnums=tuple(range(n_in, n_in + n_out)),
        keep_unused=True,
    )
    sharding = NamedSharding(mesh, PartitionSpec("core"))
    zero_shapes = [(N_CORES * a.shape[0], *a.shape[1:]) for a in out_avals]
    zeros_fn = jax.jit(
        lambda: tuple(jnp.zeros(s, np.float32) for s in zero_shapes),
        out_shardings=tuple(sharding for _ in zero_shapes),
    )
    _RUNNER = (fn, in_names, out_names, sharding, zeros_fn)
    return _RUNNER


def _run_fast(**inputs):
    import jax

    fn, in_names, out_names, sharding, zeros_fn = _get_runner()
    x = np.ascontiguousarray(inputs["x"], dtype=np.float32)
    # global [N_CORES*2048, 3136] view == the concat of the per-core shards
    global_in = {"x": x.reshape(N_CORES * N_PER_CORE * C, S)}
    for k in ("w1", "b1", "gn1_w", "gn1_b", "w2", "b2", "gn2_w", "gn2_b"):
        v = np.ascontiguousarray(inputs[k], dtype=np.float32)
        global_in[k] = np.tile(v, (N_CORES,) + (1,) * (v.ndim - 1))
    dev_in = [jax.device_put(global_in[nm], sharding) for nm in in_names]
    outs = fn(*dev_in, *zeros_fn())
    out_arr = outs[out_names.index("out")]
    # async per-shard fetch pipelines the tunnel (16x faster than a blocking
    # np.asarray of the global sharded array)
    shards = list(out_arr.addressable_shards)
    for s in shards:
        s.data.copy_to_host_async()
    out = np.empty((N_CORES * N_PER_CORE * C, S), np.float32)
    for s in shards:
        out[s.index] = np.asarray(s.data)
    return out.reshape(32, C, 56, 56)


def kernel(**inputs) -> np.ndarray:
    from concourse._compat import axon_active
    if not axon_active():
        # native (non-axon) environment: use the stock SPMD dispatcher
        out, _ = _run(trace=False, **inputs)
        return out
    try:
        return _run_fast(**inputs)
    except Exception:
        # one retry for transient device/runtime hiccups; the dispatch is
        # stateless (fresh on-device zero output buffers per call)
        return _run_fast(**inputs)

